# revision 18
# baseline (speedup 1.0000x reference)
"""Causal single-head attention (B=4, S=4096, E=1024, H=64) on 8 trn2 cores.

Sharding: core j handles batch j//2, query parity p=j%2 (256-row query
blocks interleaved by parity). Host permutes the batch's rows by 256-blocks
(pos 2m <- block 2m+p, pos 2m+1 <- block 2m+1-p) so every core runs the
same static program: query slot k = permuted rows [512k, 512k+256), its
causal kv set = permuted rows [0, 512k+512) with a fixed triangular mask on
the first half of the diagonal 512-chunk and a per-core constant mask
(input data) on the second half.

On-device dataflow per core:
  - emb arrives host-permuted, bf16, and host-transposed to [E, S] so embT
    [E-chunk 128, s-chunk 512] tiles load as plain contiguous DMAs.
  - QK.T = [WqT|WkT] @ embT (PSUM f32, PE bf16), V.T = WvT @ embT.
  - V.T is PE-transposed into natural V' [kv, 65] tiles with a ones column
    (col 64) so the PV matmul also produces the softmax denominator.
  - scores.T[kv, q] = K.T-chunk.T @ Q.T-slot (f32r), masked via DVE adds,
    exp via ScalarE (scale=1/8 folded in), PV accumulates O.T [65, 256].
  - O.T is PE-transposed to [q, 65]; q-rows scaled by reciprocal of col 64.
"""

import sys

sys.path.insert(0, "/opt/trn_rl_repo")

import numpy as np
import ml_dtypes

import concourse.bass as bass
import concourse.mybir as mybir
import concourse.tile as tile
from concourse import bacc
from concourse.bass_utils import run_bass_kernel_spmd

B, S, E, H = 4, 4096, 1024, 64
P = 128
NE = E // P  # 8 e-chunks
SC = 512  # s-chunk (proj streaming granularity)
NS = S // SC  # 8 s-chunks
QB = 256  # query block (slot) size
NSLOT = S // (2 * QB)  # 8 slots per core
NKV = S // P  # 32 kv tiles
NEG = -10000.0
F32 = mybir.dt.float32
F32R = mybir.dt.float32r
BF16 = mybir.dt.bfloat16

_CACHE = {}


def _r(ap):
    return ap.bitcast(F32R)


def _build_program():
    nc = bacc.Bacc("TRN2", target_bir_lowering=False, debug=False, num_devices=8)
    emb = nc.declare_dram_parameter("emb", [P, NS, NE, SC], BF16, isOutput=False)
    wqk = nc.declare_dram_parameter("wqk", [NE, P, P], BF16, isOutput=False)
    wv = nc.declare_dram_parameter("wv", [NE, P, H], BF16, isOutput=False)
    trimask = nc.declare_dram_parameter("trimask", [P, 2 * QB], F32, isOutput=False)
    m2mask = nc.declare_dram_parameter("m2mask", [P, 2 * QB], F32, isOutput=False)
    ident = nc.declare_dram_parameter("ident", [P, P], F32R, isOutput=False)
    ones = nc.declare_dram_parameter("ones", [P, NKV, 2], BF16, isOutput=False)
    out = nc.declare_dram_parameter("out", [2 * NSLOT, P, H], F32, isOutput=True)

    with tile.TileContext(nc) as tc:
        with (
            tc.tile_pool(name="persist", bufs=1) as pers,
            tc.tile_pool(name="embt", bufs=3) as embtp,
            tc.tile_pool(name="stage", bufs=2) as stage,
            tc.tile_pool(name="upool", bufs=3) as upool,
            tc.tile_pool(name="osmall", bufs=4) as osmall,
            tc.tile_pool(name="pp", bufs=2, space="PSUM") as pp,
            tc.tile_pool(name="scp", bufs=2, space="PSUM") as scp,
            tc.tile_pool(name="oaccp", bufs=2, space="PSUM") as oaccp,
        ):
            # ---- constants ----
            wqk_sb = pers.tile([P, NE, P], BF16, tag="wqk")
            nc.sync.dma_start(wqk_sb[:], wqk[:].rearrange("c p f -> p c f"))
            wv_sb = pers.tile([P, NE, H], BF16, tag="wv")
            nc.sync.dma_start(wv_sb[:], wv[:].rearrange("c p f -> p c f"))
            tri_sb = pers.tile([P, 2 * QB], F32, tag="tri")
            nc.sync.dma_start(tri_sb[:], trimask[:])
            m2_sb = pers.tile([P, 2 * QB], F32, tag="m2")
            nc.sync.dma_start(m2_sb[:], m2mask[:])
            id_sb = pers.tile([P, P], F32R, tag="ident")
            nc.sync.dma_start(id_sb[:], ident[:])

            warm = pers.tile([P, 1], F32, tag="warm")
            nc.scalar.activation(
                warm[:], tri_sb[:, 0:1], mybir.ActivationFunctionType.Exp, scale=0.125
            )
            ktsb = pers.tile([H, S], BF16, tag="kt")
            qtsb = pers.tile([H, NSLOT * QB], BF16, tag="qt")
            vsb = pers.tile([P, NKV, 72], BF16, tag="v")
            nc.sync.dma_start(vsb[:, :, H : H + 2], ones[:])

            def load_chunk(m):
                et = embtp.tile([P, NE, SC], BF16, tag="embt")
                nc.sync.dma_start(out=et[:, 0 : NE // 2, :], in_=emb[:, m, 0 : NE // 2])
                nc.sync.dma_start(out=et[:, NE // 2 : NE, :], in_=emb[:, m, NE // 2 : NE])
                return [et[:, e, :] for e in range(NE)]

            def proj_chunk(m, ets):
                qk = pp.tile([P, SC], F32, tag="pp")
                for e in range(NE):
                    nc.tensor.matmul(
                        qk[:],
                        wqk_sb[:, e, :],
                        ets[e],
                        start=(e == 0),
                        stop=(e == NE - 1),
                    )
                nc.vector.tensor_copy(qtsb[:, m * QB : (m + 1) * QB], qk[0:H, 0:QB])
                nc.vector.tensor_copy(ktsb[:, m * SC : (m + 1) * SC], qk[H:P, :])
                for t in range(SC // P):
                    vn = pp.tile([P, H], F32, tag="pp")
                    for e in range(NE):
                        nc.tensor.matmul(
                            vn[:],
                            ets[e][:, t * P : (t + 1) * P],
                            wv_sb[:, e, :],
                            start=(e == 0),
                            stop=(e == NE - 1),
                        )
                    nc.vector.tensor_copy(vsb[:, m * (SC // P) + t, 0:H], vn[:])

            def scores_pair(h, g, q_rhs, kind):
                # kind: "full" = 2 kv-tiles x 512q; "diag0"/"diag1" = k1 diagonal, 256q
                if kind == "full":
                    sc_t = scp.tile([P, 2, 2 * QB], F32, tag="sc")
                    for j in range(2):
                        tkv = 2 * g + j
                        nc.tensor.matmul(
                            sc_t[:, j, :],
                            ktsb[:, tkv * P : (tkv + 1) * P],
                            q_rhs,
                            start=True,
                            stop=True,
                        )
                    if g == 4 * h:  # k0 diagonal: tri masks on left half
                        nc.vector.tensor_add(
                            sc_t[:, 0, 0:QB], sc_t[:, 0, 0:QB], tri_sb[:, 0:QB]
                        )
                        nc.vector.tensor_add(
                            sc_t[:, 1, 0:QB], sc_t[:, 1, 0:QB], tri_sb[:, QB : 2 * QB]
                        )
                    if g == 4 * h + 1:  # k0 m2 on left half
                        nc.vector.tensor_add(
                            sc_t[:, 0:2, 0:QB],
                            sc_t[:, 0:2, 0:QB],
                            m2_sb[:].rearrange("p (a b) -> p a b", a=2),
                        )
                else:
                    sc_t = scp.tile([P, 2, QB], F32, tag="sc")
                    for j in range(2):
                        tkv = 2 * g + j
                        nc.tensor.matmul(
                            sc_t[:, j, :],
                            ktsb[:, tkv * P : (tkv + 1) * P],
                            q_rhs[:, QB : 2 * QB],
                            start=True,
                            stop=True,
                        )
                    if kind == "diag0":
                        nc.vector.tensor_add(sc_t[:, 0, :], sc_t[:, 0, :], tri_sb[:, 0:QB])
                        nc.vector.tensor_add(
                            sc_t[:, 1, :], sc_t[:, 1, :], tri_sb[:, QB : 2 * QB]
                        )
                    else:
                        nc.vector.tensor_add(
                            sc_t[:, 0:2, :],
                            sc_t[:, 0:2, :],
                            m2_sb[:].rearrange("p (a b) -> p a b", a=2),
                        )
                u = upool.tile([P, 2, 2 * QB], BF16, tag="u")
                w = 2 * QB if kind == "full" else QB
                nc.scalar.activation(
                    u[:, :, 0:w],
                    sc_t[:],
                    mybir.ActivationFunctionType.Exp,
                    scale=0.125,
                )
                return u

            def pv_pair(h, g, ot, u, kind, first, last):
                for j in range(2):
                    tkv = 2 * g + j
                    if kind == "full":
                        nc.tensor.matmul(
                            ot[:],
                            vsb[:, tkv, 0 : H + 2],
                            u[:, j, :],
                            start=(first and j == 0),
                            stop=(last and j == 1),
                        )
                    else:
                        nc.tensor.matmul(
                            ot[:, QB : 2 * QB],
                            vsb[:, tkv, 0 : H + 2],
                            u[:, j, 0:QB],
                            start=False,
                            stop=(last and j == 1),
                        )

            def finalize_pair(h, otsb):
                for h2 in range(4):
                    ott = pp.tile([P, H + 2], F32, tag="pp")
                    nc.tensor.transpose(
                        _r(ott[:]),
                        otsb[:, h2 * P : (h2 + 1) * P],
                        id_sb[0 : H + 2, 0 : H + 2],
                    )
                    rec = osmall.tile([P, 1], F32, tag="rec")
                    nc.vector.reciprocal(rec[:], ott[:, H : H + 1])
                    o_t = osmall.tile([P, H], F32, tag="o")
                    nc.vector.tensor_scalar_mul(o_t[:], ott[:, 0:H], rec[:])
                    nc.gpsimd.dma_start(out=out[4 * h + h2], in_=o_t[:])

            ets0 = load_chunk(0)
            ets1 = load_chunk(1)
            proj_chunk(0, ets0)
            proj_chunk(1, ets1)
            pend = None
            for h in range(4):
                nxt = None
                if h < 3:
                    nxt = (load_chunk(2 * h + 2), load_chunk(2 * h + 3))
                q_rhs = qtsb[:, 2 * h * QB : (2 * h + 2) * QB]
                ot = oaccp.tile([H + 2, 2 * QB], F32, tag="ot")
                # group schedule: shared 2-tile groups 0..4h+1, then diag0, diag1
                glist = [(g, "full") for g in range(4 * h + 2)]
                glist += [(4 * h + 2, "diag0"), (4 * h + 3, "diag1")]
                u_cur = scores_pair(h, glist[0][0], q_rhs, glist[0][1])
                if pend is not None:
                    finalize_pair(pend[0], pend[1])
                    pend = None
                for idx in range(len(glist)):
                    if idx + 1 < len(glist):
                        u_next = scores_pair(h, glist[idx + 1][0], q_rhs, glist[idx + 1][1])
                    else:
                        u_next = None
                        if nxt is not None:
                            proj_chunk(2 * h + 2, nxt[0])
                            proj_chunk(2 * h + 3, nxt[1])
                    g, kind = glist[idx]
                    pv_pair(h, g, ot, u_cur, kind, idx == 0, idx == len(glist) - 1)
                    u_cur = u_next
                otsb = osmall.tile([H + 2, 2 * QB], F32R, tag="otsb")
                nc.vector.tensor_copy(otsb[:], ot[:])
                pend = (h, otsb)
            finalize_pair(pend[0], pend[1])
    nc.compile()
    return nc


def _host_inputs(embeddings, W_Q, W_K, W_V):
    """Build the 8 per-core input maps."""
    wqk = np.empty((NE, P, P), np.float32)
    wv = np.empty((NE, P, H), np.float32)
    for c in range(NE):
        wqk[c, :, 0:H] = W_Q[:, c * P : (c + 1) * P].T
        wqk[c, :, H:P] = W_K[:, c * P : (c + 1) * P].T
        wv[c] = W_V[:, c * P : (c + 1) * P].T
    wqk = wqk.astype(ml_dtypes.bfloat16)
    wv = wv.astype(ml_dtypes.bfloat16)

    ki = np.arange(P)[:, None]
    qj = np.arange(QB)[None, :]
    trimask = np.zeros((P, 2 * QB), np.float32)
    trimask[:, 0:QB] = np.where(qj >= ki, 0.0, NEG)
    trimask[:, QB : 2 * QB] = np.where(qj >= ki + P, 0.0, NEG)
    ident = np.eye(P, dtype=np.float32)

    in_maps = []
    for j in range(8):
        b, p = j // 2, j % 2
        eb = embeddings[b].reshape(S // QB, QB, E)
        order = np.empty(S // QB, np.int64)
        for m in range(S // (2 * QB)):
            order[2 * m] = 2 * m + p
            order[2 * m + 1] = 2 * m + 1 - p
        embp = np.ascontiguousarray(
            eb[order]
            .reshape(S, E)
            .astype(ml_dtypes.bfloat16)
            .T.reshape(NE, P, NS, SC)
            .transpose(1, 2, 0, 3)
        )
        m2 = np.full((P, 2 * QB), NEG if p == 0 else 0.0, np.float32)
        in_maps.append(
            {
                "emb": embp,
                "wqk": wqk,
                "wv": wv,
                "trimask": trimask,
                "m2mask": m2,
                "ident": ident,
                "ones": np.ones((P, NKV, 2), ml_dtypes.bfloat16),
            }
        )
    return in_maps


def _assemble(results):
    out = np.empty((B, S, H), np.float32)
    for j in range(8):
        b, p = j // 2, j % 2
        o = results[j]["out"]  # [16, 128, 64]
        for k in range(NSLOT):
            g0 = (2 * k + p) * QB
            out[b, g0 : g0 + P] = o[2 * k]
            out[b, g0 + P : g0 + 2 * P] = o[2 * k + 1]
    return out


def kernel(embeddings, W_Q, W_K, W_V, _trace=False, _tmpdir=None):
    if "nc" not in _CACHE:
        _CACHE["nc"] = _build_program()
    nc = _CACHE["nc"]
    in_maps = _host_inputs(
        np.asarray(embeddings), np.asarray(W_Q), np.asarray(W_K), np.asarray(W_V)
    )
    res = run_bass_kernel_spmd(
        nc, in_maps, list(range(8)), trace=_trace, tmpdir=_tmpdir
    )
    out = _assemble(res.results)
    if _trace:
        return out, res
    return out


if __name__ == "__main__":
    rng = np.random.default_rng(0)
    emb = rng.standard_normal((B, S, E), dtype=np.float32)
    wq = rng.uniform(-0.07, 0.07, (H, E)).astype(np.float32)
    wk = rng.uniform(-0.07, 0.07, (H, E)).astype(np.float32)
    wv_ = rng.uniform(-0.07, 0.07, (H, E)).astype(np.float32)
    o = kernel(emb, wq, wk, wv_)
    print("ok", o.shape, o.dtype)
